# revision 31
# baseline (speedup 1.0000x reference)
"""Trainium2 Bass kernel for nn_Interpolator: zero-stuff upsample x8 + 128-tap FIR (SAME) + x8 gain.

Polyphase formulation: with m indexing 64-sample rows of x and n = 8*q' + r in [0, 512),
    y[512*m + n] = sum_{k=0}^{78} T4[k, m] * H4[k, n]
where T4[k, m] = x[64*m + k - 7] (zero-padded) and
    H4[k, 8*q'+r] = 8 * h[(7-r) + 8*(k-q')]  for 0 <= k-q' <= 15, else 0.

Per core (8 cores, batch-parallel): 16 signals (8 batch rows x {real, imag}).

v2: the im2col (T4) is built on the HOST and shipped as a [79, 16*512] fp16
DRAM tensor (k-major), so the kernel needs NO xbar DMA-transposes at all.
That removes the serial xbar block + mode-transition drains that previously
kept stores from starting until ~21 us.  Loads are plain DMAs issued on the
scalar HWDGE queue; stores go on the sync queue with a PERMUTED, per-partition
contiguous DRAM layout (y_dev[sig, i, 512*t + n] = y[sig, 65536*t + 512*i + n],
AP [[2048,128],[1,2048]] = 128 x 4KB descriptors); the host un-permutes.

The PE is at 1.2 GHz on this part, so the 64 matmuls (512 cols each) are the
27.5 us critical path; loads/casts/stores pipeline behind it.  PSUM is tiled
[128, 1024] (2 banks) x 4 bufs: vector casts the t0/t1 half, scalar the t2/t3
half; the last signal splits the tail cast across both engines and stores in
two halves so the final drain starts sooner.  y is fp16 on device; the host
casts to fp32.
"""

import numpy as np
from numpy.lib.stride_tricks import sliding_window_view

import concourse.bass as bass
import concourse.tile as tile
from concourse import bacc, mybir
from concourse.bass_utils import run_bass_kernel_spmd

B = 64
N = 32768
FACTOR = 8
NOUT = N * FACTOR  # 262144
N_CORES = 8
ROWS_PER_CORE = B // N_CORES  # 8
SIGS = 2 * ROWS_PER_CORE  # 16 signals per core (real rows then imag rows)
K = 79  # contraction window length
NPAD = 32832  # 7 leading zeros + N + 57 trailing zeros; = 64*513
M = 512  # T4 columns (m-values) per signal
TILES = 4  # out tiles per signal, each [128 m-rows, 512 samples]

# Load chunks in units of 128 T4 columns (1 signal = 4 units); the first
# chunk carries H4 (512 lead cols) + sig 0's first matmul tile so the PE can
# start as early as possible.
CHUNK_UNITS = (4, 12, 16, 16, 16)  # sums to 64 = SIGS*4

_F16 = mybir.dt.float16
_F32 = mybir.dt.float32

_NC_CACHE = {}


def _build_nc():
    nc = bacc.Bacc(
        "TRN2",
        target_bir_lowering=False,
        debug=False,
        enable_asserts=False,
        num_devices=N_CORES,
    )
    # t4x holds [128 rows, 512 + SIGS*512 cols]: cols 0-511 = H4 (rows 79+ junk),
    # cols 512.. = T4 of the 16 signals (512 m-cols each)
    t4 = nc.dram_tensor("t4", [128 * (512 + SIGS * M)], _F16, kind="ExternalInput")
    y = nc.dram_tensor("y", [SIGS, NOUT], _F16, kind="ExternalOutput")

    with tile.TileContext(nc) as tc:
        with (
            tc.tile_pool(name="t4pool", bufs=len(CHUNK_UNITS)) as t4pool,
            tc.tile_pool(name="opool", bufs=8) as opool,
            tc.tile_pool(name="po", bufs=4, space="PSUM") as po_pool,
        ):
            # per-(sig, t) map to (tile, local col base) after its chunk's load
            t4_of_tile = {}
            h4_sb = None  # set by the first chunk load (h4 rides along)
            DSTRIDE = 512 + SIGS * M  # row stride of t4x in DRAM

            def load_chunk(first_unit, n_units):
                nonlocal h4_sb
                lead = 512 if first_unit == 0 else 0  # h4 rides in chunk 0
                w = 128 * n_units + lead
                T4g = t4pool.tile([128, w], _F16, tag="t4")
                nc.sync.dma_start(
                    out=T4g[:, :],
                    in_=bass.AP(
                        tensor=t4,
                        offset=512 + first_unit * 128 - lead,
                        ap=[[DSTRIDE, 128], [1, w]],
                    ),
                )
                if first_unit == 0:
                    h4_sb = T4g
                for u in range(n_units):
                    g = first_unit + u
                    t4_of_tile[(g // 4, g % 4)] = (T4g, 128 * u + lead)

            def store_cols(sig, out_sb, c0, c1):
                nc.sync.dma_start(
                    out=bass.AP(
                        tensor=y,
                        offset=sig * NOUT + c0,
                        ap=[[2048, 128], [1, c1 - c0]],
                    ),
                    in_=out_sb[:, c0:c1],
                )

            def compute_store(sig):
                """4 matmuls -> 2x [128,1024] PSUM, two half casts, one 512 KB store."""
                last = sig == SIGS - 1
                out_sb = opool.tile([128, TILES * 512], _F16)
                for half in range(2):
                    po = po_pool.tile([128, 1024], _F32, tag="po")
                    for s in range(2):
                        t = 2 * half + s
                        T4g, base = t4_of_tile[(sig, t)]
                        nc.tensor.matmul(
                            po[:, 512 * s : 512 * (s + 1)],
                            T4g[0:K, base : base + 128],
                            h4_sb[0:K, 0:512],
                            start=True,
                            stop=True,
                        )
                        if last and half == 1 and s == 0:
                            # last signal: cast+store MM t2's bank while MM t3
                            # still streams (different PSUM bank, no conflict)
                            nc.scalar.copy(out=out_sb[:, 1024:1536], in_=po[:, 0:512])
                            store_cols(sig, out_sb, 1024, 1536)
                    if half == 0:
                        nc.vector.tensor_copy(out=out_sb[:, 0:1024], in_=po[:, :])
                        if last:
                            store_cols(sig, out_sb, 0, 1024)
                    elif last:
                        # final tile: split the cast across both engines, then a
                        # small final store so the drain (receipt) starts ASAP
                        nc.scalar.copy(out=out_sb[:, 1536:1792], in_=po[:, 512:768])
                        nc.vector.tensor_copy(
                            out=out_sb[:, 1792:2048], in_=po[:, 768:1024]
                        )
                        store_cols(sig, out_sb, 1536, 2048)
                    else:
                        nc.scalar.copy(out=out_sb[:, 1024:2048], in_=po[:, :])
                # store: y_dev[sig, i, c] = out_sb[i, c]  (per-partition 4KB contig)
                if not last:
                    store_cols(sig, out_sb, 0, 2048)

            first = 0
            for g in CHUNK_UNITS:
                load_chunk(first, g)
                first += g
            for sig in range(SIGS):
                compute_store(sig)

    nc.compile()
    return nc


def _get_nc():
    if "nc" not in _NC_CACHE:
        _NC_CACHE["nc"] = _build_nc()
    return _NC_CACHE["nc"]


def _build_h4(h):
    h4 = np.zeros((K, 512), np.float32)
    qp = np.arange(64)
    for t in range(16):
        for r in range(8):
            h4[qp + t, 8 * qp + r] = FACTOR * h[(7 - r) + 8 * t]
    return h4


def _run(x_real, x_imag, fir_filter, trace=False):
    h4 = _build_h4(np.asarray(fir_filter, np.float32)).astype(np.float16)
    # host-side im2col for all 128 signals: T4[k, m] = x_pad[64m + k]
    xpad = np.zeros((2, B, NPAD), np.float16)
    xpad[0, :, 7 : 7 + N] = x_real
    xpad[1, :, 7 : 7 + N] = x_imag
    # windows[part, b, m, k] = xpad[part, b, 64m + k]
    windows = sliding_window_view(xpad, K, axis=2)[:, :, ::64, :]  # [2, B, 512, 79]
    in_maps = []
    for c in range(N_CORES):
        rows = slice(c * ROWS_PER_CORE, (c + 1) * ROWS_PER_CORE)
        # t4c[k, 512 + 512*s + m], signals = 8 real rows then 8 imag rows;
        # cols 0-511 carry H4 so chunk 0 delivers weights + sig 0 in one DMA
        t4c = np.zeros((128, 512 + SIGS * M), np.float16)
        t4c[:K, :512] = h4
        t4c[:K, 512:] = (
            windows[:, rows].reshape(SIGS, M, K).transpose(2, 0, 1).reshape(K, -1)
        )
        in_maps.append({"t4": t4c.reshape(-1)})
    nc = _get_nc()
    res = run_bass_kernel_spmd(nc, in_maps, core_ids=list(range(N_CORES)), trace=trace)
    out = np.empty((2, B, NOUT), np.float32)
    for c in range(N_CORES):
        yc = res.results[c]["y"]
        # y_dev[sig, i, 512t + n] = y[sig, 65536t + 512i + n]
        yc = yc.reshape(SIGS, 128, TILES, 512).transpose(0, 2, 1, 3).reshape(SIGS, NOUT)
        rows = slice(c * ROWS_PER_CORE, (c + 1) * ROWS_PER_CORE)
        out[0, rows] = yc[:ROWS_PER_CORE]
        out[1, rows] = yc[ROWS_PER_CORE:]
    return out, res


def kernel(x_real, x_imag, fir_filter, factor):
    assert int(factor) == FACTOR
    x_real = np.asarray(x_real, np.float32)
    x_imag = np.asarray(x_imag, np.float32)
    assert x_real.shape == (B, N) and x_imag.shape == (B, N)
    out, _ = _run(x_real, x_imag, fir_filter)
    return out
